# revision 15
# baseline (speedup 1.0000x reference)
"""BotGAT Trainium2 kernel: 8-core node-sharded GAT, dma_gather edge phase.

Table row per node (bf16, 128 values = 256B): r = T @ h where T is an
invertible per-head block transform whose last two rows per head are a_src_h
and a_dst_h.  So a gathered src row carries als at col h*C+(C-2), a gathered
dst row carries ald at col h*C+(C-1), and T^{-1} folds into the next layer's
weights on the host.

Edge phase per chunk (a few dst tiles):
  G  = dma_gather(table[range], idx16)   x4 ranges   [128, J, 128] bf16
  Gd = dma_gather(tb_loc, dst idx16)                 [128, J, 128] bf16
  ef = lrelu(als + ald, 0.2); ex = exp(ef)           (scalar)
  GW[:, :, 0:128] = G * ex  ; GW[:, :, 128:132] = ex (vector)
  per tile: M = one-hot(dstloc)  (vector IS_EQ)
            psum[s, 0:132] += M_b.T @ GW_b  per column (tensor)
            x' = psum[:, 0:128] / psum[:, 128+h]  -> x_sb  (vector)
  transpose sweep x_sb -> x'T slab (tensor+scalar)
"""
import sys
sys.path.insert(0, "/opt/trn_rl_repo")
import numpy as np
from dataclasses import dataclass
from contextlib import ExitStack

from concourse import bass, bacc, mybir, tile

F32 = mybir.dt.float32
BF16 = mybir.dt.bfloat16
I32 = mybir.dt.int32
I16 = mybir.dt.int16
P = 128

N_REAL = 100000
NCORES = 8
TPC = 98
NPC = TPC * P            # 12544
NPAD = NCORES * NPC      # 100352
NRANGE = 4
RSZ = NPAD // NRANGE     # 25088 <= 32767 ok for int16
JBUDGET = 80             # max gather j-columns per chunk


def _segs(n, maxseg=512):
    out = []
    while n > 0:
        s = min(maxseg, n)
        out.append(s); n -= s
    return out


# ---------------------------------------------------------------------------
# structure = (nb[t][r] tuples, chunks as (t0, t1) pairs)
def make_structure(nb):
    # nb: [TPC, NRANGE] ints (max over cores of per-(tile,range) col counts)
    chunks = []
    t0 = 0
    while t0 < TPC:
        t1 = t0
        tot = 0
        while t1 < TPC and tot + sum(nb[t1]) <= JBUDGET:
            tot += sum(nb[t1]); t1 += 1
        assert t1 > t0, f"tile {t0} alone exceeds JBUDGET"
        chunks.append((t0, t1))
        t0 = t1
    return tuple(tuple(r) for r in nb), tuple(chunks)


def col_layout(nbs, chunks):
    """Global G-column order: chunk-major, then range, then tile, then j.
    Returns: gcol[t][r] = list of global col indices;
             chunk_cols[ci] = (start, ncols); per-chunk range group spans."""
    gcol = [[None] * NRANGE for _ in range(TPC)]
    chunk_info = []
    pos = 0
    for (t0, t1) in chunks:
        start = pos
        rspans = []
        for r in range(NRANGE):
            rs = pos
            for t in range(t0, t1):
                n = nbs[t][r]
                gcol[t][r] = list(range(pos, pos + n))
                pos += n
            rspans.append((rs, pos))
        chunk_info.append((start, pos - start, rspans))
    return gcol, chunk_info, pos  # pos == JCOLS


# tile-major layout for dstloc/M: per tile, its cols contiguous
def tile_layout(nbs):
    tcol = []
    pos = 0
    for t in range(TPC):
        n = sum(nbs[t])
        tcol.append((pos, n))
        pos += n
    return tcol, pos


# ---------------------------------------------------------------------------
def build(struct):
    nbs, chunks = struct
    gcol, chunk_info, JCOLS = col_layout(nbs, chunks)
    tcol, JT = tile_layout(nbs)
    assert JT == JCOLS

    nc = bacc.Bacc("TRN2", target_bir_lowering=False, debug=False,
                   num_devices=NCORES)
    segs = _segs(NPC)

    inp = {}
    def di(name, shape, dt):
        inp[name] = nc.dram_tensor(name, list(shape), dt, kind="ExternalInput")

    di("desT", [768, NPC], BF16); di("tweetT", [768, NPC], BF16)
    di("numT", [5, NPC], BF16);   di("catT", [3, NPC], BF16)
    di("W_des", [768, 32], BF16); di("W_tw", [768, 32], BF16)
    di("W_np", [5, 32], BF16);    di("W_cp", [3, 32], BF16)
    di("b_enc", [128, 1], F32)
    di("W_in", [128, 128], BF16); di("b_in", [128, 1], F32)
    di("gat1_W", [128, 128], BF16)
    di("gat2_W", [128, 128], BF16); di("c2", [128, 1], F32)
    di("W_o1", [128, 128], BF16); di("b_o1", [128, 1], F32)
    di("W_o2", [128, 2], BF16);   di("b_o2", [2, 1], F32)
    di("iota_bf", [P, P], BF16)
    di("ident_bf", [P, P], BF16)
    di("idxw_src", [P, JCOLS * 8], I16)
    di("idxw_dst", [P, JCOLS * 8], I16)
    di("dstloc", [P, JCOLS], BF16)   # tile-major order

    outp = nc.dram_tensor("outT", [2, NPC], F32, kind="ExternalOutput")

    tb_loc = [nc.dram_tensor(f"tb_loc{l}", [NPC, P], BF16) for l in (1, 2)]
    table = [nc.dram_tensor(f"table{l}", [NPAD, P], BF16) for l in (1, 2)]

    with tile.TileContext(nc) as tc:
      with ExitStack() as top:
        consts = top.enter_context(tc.tile_pool(name="consts", bufs=1))
        slabs = top.enter_context(tc.tile_pool(name="slabs", bufs=2))

        def big():
            return slabs.tile([128, NPC], BF16, tag="big", name="bigslab")

        iota_t = consts.tile([P, P], BF16, tag="iota")
        nc.sync.dma_start(out=iota_t[:], in_=inp["iota_bf"][:, :])
        identb_t = consts.tile([P, P], BF16, tag="identb")
        nc.sync.dma_start(out=identb_t[:], in_=inp["ident_bf"][:, :])
        smallw = {}
        for name, shape, dt in [
                ("W_in", (128, 128), BF16), ("gat1_W", (128, 128), BF16),
                ("gat2_W", (128, 128), BF16), ("W_o1", (128, 128), BF16),
                ("W_o2", (128, 2), BF16),
                ("b_enc", (128, 1), F32), ("b_in", (128, 1), F32),
                ("c2", (128, 1), F32), ("b_o1", (128, 1), F32),
                ("b_o2", (2, 1), F32)]:
            t = consts.tile(list(shape), dt, tag=f"c_{name}")
            nc.sync.dma_start(out=t[:], in_=inp[name][:, :])
            smallw[name] = t
        dstloc_t = consts.tile([P, JCOLS], BF16, tag="dstloc")
        nc.sync.dma_start(out=dstloc_t[:], in_=inp["dstloc"][:, :])

        # =================== encoders -> xT ===================
        xT = big()
        with ExitStack() as ph:
            lp = ph.enter_context(tc.tile_pool(name="enc_load", bufs=3))
            wp = ph.enter_context(tc.tile_pool(name="enc_w", bufs=1))
            pp = ph.enter_context(tc.tile_pool(name="enc_psum", bufs=4, space="PSUM"))

            mods = [("desT", "W_des", 768, 0), ("tweetT", "W_tw", 768, 32),
                    ("numT", "W_np", 5, 64), ("catT", "W_cp", 3, 96)]
            wts = {}
            for mod, wn, K, _ in mods:
                nk = (K + 127) // 128
                w = wp.tile([min(K, 128), nk * 32], BF16, tag=f"w_{wn}")
                for kc in range(nk):
                    k0, k1 = kc * 128, min(K, (kc + 1) * 128)
                    nc.sync.dma_start(out=w[0:k1 - k0, kc * 32:(kc + 1) * 32],
                                      in_=inp[wn][k0:k1, :])
                wts[mod] = (w, nk, K)

            off = 0
            for seg in segs:
                for mod, wn, K, pbase in mods:
                    w, nk, K = wts[mod]
                    ps = pp.tile([32, 512], F32, space="PSUM", tag="enc_ps")
                    for kc in range(nk):
                        k0, k1 = kc * 128, min(K, (kc + 1) * 128)
                        rt = lp.tile([128, 512], BF16, tag="enc_rhs")
                        nc.sync.dma_start(out=rt[0:k1 - k0, 0:seg],
                                          in_=inp[mod][k0:k1, off:off + seg])
                        nc.tensor.matmul(out=ps[:, 0:seg],
                                         lhsT=w[0:k1 - k0, kc * 32:(kc + 1) * 32],
                                         rhs=rt[0:k1 - k0, 0:seg],
                                         start=(kc == 0), stop=(kc == nk - 1))
                    nc.scalar.activation(out=xT[pbase:pbase + 32, off:off + seg],
                                         in_=ps[:, 0:seg],
                                         func=mybir.ActivationFunctionType.Lrelu,
                                         bias=smallw["b_enc"][pbase:pbase + 32, :],
                                         alpha=0.01)
                off += seg

        # =================== uT ===================
        uT = big()
        with ExitStack() as ph:
            pp = ph.enter_context(tc.tile_pool(name="nd_psum", bufs=4, space="PSUM"))
            off = 0
            for seg in segs:
                ps = pp.tile([128, 512], F32, space="PSUM", tag="nd_ps")
                nc.tensor.matmul(out=ps[:, 0:seg], lhsT=smallw["W_in"][:, :],
                                 rhs=xT[:, off:off + seg], start=True, stop=True)
                nc.scalar.activation(out=uT[:, off:off + seg], in_=ps[:, 0:seg],
                                     func=mybir.ActivationFunctionType.Lrelu,
                                     bias=smallw["b_in"][:, :], alpha=0.01)
                off += seg

        # ============ per-layer table build + edge phase ===============
        def build_table(srcT, gwn, biasn, lix):
            gw = smallw[gwn]
            hT = big()
            with ExitStack() as ph:
                pp = ph.enter_context(tc.tile_pool(name="tb_psum", bufs=4, space="PSUM"))
                off = 0
                for seg in segs:
                    ps = pp.tile([128, 512], F32, space="PSUM", tag="tb_ps")
                    nc.tensor.matmul(out=ps[:, 0:seg], lhsT=gw[:, :],
                                     rhs=srcT[:, off:off + seg], start=True, stop=True)
                    if biasn is None:
                        nc.scalar.copy(out=hT[:, off:off + seg], in_=ps[:, 0:seg])
                    else:
                        nc.scalar.activation(
                            out=hT[:, off:off + seg], in_=ps[:, 0:seg],
                            func=mybir.ActivationFunctionType.Identity,
                            bias=smallw[biasn][:, :])
                    off += seg

            GB = 8
            with ExitStack() as ph:
                ap_ = ph.enter_context(tc.tile_pool(name="asm", bufs=2))
                pp = ph.enter_context(tc.tile_pool(name="asm_psum", bufs=3, space="PSUM"))
                for g0 in range(0, TPC, GB):
                    gn = min(GB, TPC - g0)
                    asm = ap_.tile([P, GB * P], BF16, tag="asm_t")
                    for j in range(gn):
                        n0 = (g0 + j) * P
                        ph_ = pp.tile([P, P], BF16, space="PSUM", tag="asm_ph")
                        nc.tensor.transpose(out=ph_[:], in_=hT[:, n0:n0 + P],
                                            identity=identb_t[:])
                        nc.scalar.copy(out=asm[:, j * P:(j + 1) * P], in_=ph_[:])
                    dv = tb_loc[lix][g0 * P:(g0 + gn) * P, :].rearrange(
                        "(b p) c -> p b c", p=P)
                    nc.sync.dma_start(out=dv, in_=asm[:, 0:gn * P].rearrange(
                        "p (b c) -> p b c", b=gn))

            nc.gpsimd.collective_compute(
                "AllGather", mybir.AluOpType.bypass,
                replica_groups=[list(range(NCORES))],
                ins=[tb_loc[lix][:, :].opt()],
                outs=[table[lix][:, :].opt()],
            )

        def edge_phase(H, lix):
            C = 128 // H
            MAXTN = max(sum(row) for row in nbs)
            x_sb = big()
            with ExitStack() as ph:
                ip_ = ph.enter_context(tc.tile_pool(name="ichunk", bufs=2))
                gp = ph.enter_context(tc.tile_pool(name="gchunk", bufs=2))
                gdp = ph.enter_context(tc.tile_pool(name="gdchunk", bufs=2))
                ep = ph.enter_context(tc.tile_pool(name="extiles", bufs=2))
                mp = ph.enter_context(tc.tile_pool(name="mtiles", bufs=2))
                gwp = ph.enter_context(tc.tile_pool(name="gwtiles", bufs=2))
                pp = ph.enter_context(tc.tile_pool(name="acc_psum", bufs=2, space="PSUM"))
                sp = ph.enter_context(tc.tile_pool(name="small", bufs=4))

                for ci, ((t0, t1), (cst, ncols, rspans)) in enumerate(
                        zip(chunks, chunk_info)):
                    isrc = ip_.tile([P, JBUDGET * 8], I16, tag="isrc")
                    nc.sync.dma_start(out=isrc[:, 0:ncols * 8],
                                      in_=inp["idxw_src"][:, cst * 8:(cst + ncols) * 8])
                    idst = ip_.tile([P, JBUDGET * 8], I16, tag="idst")
                    nc.sync.dma_start(out=idst[:, 0:ncols * 8],
                                      in_=inp["idxw_dst"][:, cst * 8:(cst + ncols) * 8])

                    G = gp.tile([P, JBUDGET, P], BF16, tag="G")
                    for r, (rs, re) in enumerate(rspans):
                        nj = re - rs
                        if nj == 0:
                            continue
                        nc.gpsimd.dma_gather(
                            G[:, rs - cst:re - cst, :],
                            table[lix][r * RSZ:(r + 1) * RSZ, :],
                            isrc[:, (rs - cst) * 8:(re - cst) * 8],
                            nj * P, nj * P, P, single_packet=False)
                    Gd = gdp.tile([P, JBUDGET, P], BF16, tag="Gd")
                    nc.gpsimd.dma_gather(
                        Gd[:, 0:ncols, :], tb_loc[lix][:, :],
                        idst[:, 0:ncols * 8], ncols * P, ncols * P, P,
                        single_packet=False)

                    # ef = als + ald (strided col extraction), lrelu, exp
                    ef = ep.tile([P, JBUDGET * 4], F32, tag="ef")
                    efv = ef[:].rearrange("p (b r) -> p b r", b=JBUDGET)[:, 0:ncols, 0:H]
                    nc.vector.tensor_tensor(
                        out=efv,
                        in0=G[:, 0:ncols, :].rearrange(
                            "p b (h c) -> p b h c", h=H)[:, :, :, C - 2],
                        in1=Gd[:, 0:ncols, :].rearrange(
                            "p b (h c) -> p b h c", h=H)[:, :, :, C - 1],
                        op=mybir.AluOpType.add)
                    nc.scalar.activation(out=efv, in_=efv,
                                         func=mybir.ActivationFunctionType.Lrelu,
                                         alpha=0.2)
                    exb = ep.tile([P, JBUDGET * 4], BF16, tag="exb")
                    exv = exb[:].rearrange("p (b r) -> p b r", b=JBUDGET)[:, 0:ncols, 0:H]
                    nc.scalar.activation(out=exv, in_=efv,
                                         func=mybir.ActivationFunctionType.Exp)

                    GW = gwp.tile([P, JBUDGET, 132], BF16, tag="GW")
                    nc.vector.tensor_tensor(
                        out=GW[:, 0:ncols, 0:128].rearrange(
                            "p b (h c) -> p b h c", h=H),
                        in0=G[:, 0:ncols, :].rearrange("p b (h c) -> p b h c", h=H),
                        in1=exb[:].rearrange("p (b r) -> p b r", b=JBUDGET)[
                            :, 0:ncols, 0:H].unsqueeze(3).to_broadcast(
                            [P, ncols, H, C]),
                        op=mybir.AluOpType.mult)
                    nc.vector.tensor_copy(
                        out=GW[:, 0:ncols, 128:128 + H],
                        in_=exb[:].rearrange("p (b r) -> p b r", b=JBUDGET)[
                            :, 0:ncols, 0:H])
                    if H < 4:
                        nc.vector.memset(GW[:, 0:ncols, 128 + H:132], 0.0)

                    for t in range(t0, t1):
                        ts, tn = tcol[t]
                        M = mp.tile([P, MAXTN * P], BF16, tag="M")
                        nc.vector.tensor_tensor(
                            out=M[:].rearrange("p (b s) -> p b s", b=MAXTN)[
                                :, 0:tn, :],
                            in0=iota_t[:].unsqueeze(1).to_broadcast([P, tn, P]),
                            in1=dstloc_t[:, ts:ts + tn].unsqueeze(2).to_broadcast(
                                [P, tn, P]),
                            op=mybir.AluOpType.is_equal)
                        ps = pp.tile([P, 132], F32, space="PSUM", tag="acc")
                        cols = [c for r in range(NRANGE) for c in gcol[t][r]]
                        for bi, c in enumerate(cols):
                            nc.tensor.matmul(
                                out=ps[:, :],
                                lhsT=M[:, bi * P:(bi + 1) * P],
                                rhs=GW[:, c - cst, 0:132],
                                start=(bi == 0), stop=(bi == len(cols) - 1))
                        rden = sp.tile([P, 4], F32, tag="rden")
                        nc.vector.tensor_scalar_add(out=rden[:, 0:H],
                                                    in0=ps[:, 128:128 + H],
                                                    scalar1=1e-16)
                        nc.vector.reciprocal(out=rden[:, 0:H], in_=rden[:, 0:H])
                        nc.vector.tensor_tensor(
                            out=x_sb[:, t * P:(t + 1) * P].rearrange(
                                "p (h c) -> p h c", h=H),
                            in0=ps[:, 0:128].rearrange("p (h c) -> p h c", h=H),
                            in1=rden[:, 0:H].unsqueeze(2).to_broadcast([P, H, C]),
                            op=mybir.AluOpType.mult)

            xoutT = big()
            with ExitStack() as ph:
                pp = ph.enter_context(tc.tile_pool(name="tr_psum", bufs=4, space="PSUM"))
                for t in range(TPC):
                    pt = pp.tile([P, P], BF16, space="PSUM", tag="tr_ps")
                    nc.tensor.transpose(out=pt[:], in_=x_sb[:, t * P:(t + 1) * P],
                                        identity=identb_t[:])
                    nc.scalar.copy(out=xoutT[:, t * P:(t + 1) * P], in_=pt[:])
            return xoutT

        build_table(uT, "gat1_W", None, 0)
        x1T = edge_phase(4, 0)
        build_table(x1T, "gat2_W", "c2", 1)
        x2T = edge_phase(1, 1)

        # head
        o1T = big()
        with ExitStack() as ph:
            pp = ph.enter_context(tc.tile_pool(name="hd_psum", bufs=4, space="PSUM"))
            op_ = ph.enter_context(tc.tile_pool(name="hd_out", bufs=2))
            off = 0
            for seg in segs:
                ps = pp.tile([128, 512], F32, space="PSUM", tag="hd_ps")
                nc.tensor.matmul(out=ps[:, 0:seg], lhsT=smallw["W_o1"][:, :],
                                 rhs=x2T[:, off:off + seg], start=True, stop=True)
                nc.scalar.activation(out=o1T[:, off:off + seg], in_=ps[:, 0:seg],
                                     func=mybir.ActivationFunctionType.Lrelu,
                                     bias=smallw["b_o1"][:, :], alpha=0.01)
                off += seg
            off = 0
            for seg in segs:
                ps2 = pp.tile([2, 512], F32, space="PSUM", tag="hd_ps2")
                nc.tensor.matmul(out=ps2[:, 0:seg], lhsT=smallw["W_o2"][:, :],
                                 rhs=o1T[:, off:off + seg], start=True, stop=True)
                ot = op_.tile([2, 512], F32, tag="hd_ot")
                nc.scalar.activation(out=ot[:, 0:seg], in_=ps2[:, 0:seg],
                                     func=mybir.ActivationFunctionType.Identity,
                                     bias=smallw["b_o2"][:, :])
                nc.sync.dma_start(out=outp[:, off:off + seg], in_=ot[:, 0:seg])
                off += seg

    nc.compile()
    return nc


# ---------------------------------------------------------------------------
def make_T(a_rows):
    """Invertible C x C block: rows = [orthonormal complement; a_src; a_dst].
    a_rows: [2, C] (or [k, C]). Returns T [C, C] float64."""
    a = np.asarray(a_rows, np.float64)
    k, C = a.shape
    A = np.eye(C)
    A[:, 0:k] = a.T
    Q, _ = np.linalg.qr(A)
    comp = Q[:, k:]            # C-k orthonormal cols spanning complement
    T = np.vstack([comp.T, a])
    return T


def prep(inputs):
    import ml_dtypes
    bf = ml_dtypes.bfloat16
    N = N_REAL
    src = np.asarray(inputs["edge_index"][0]).astype(np.int64)
    dst = np.asarray(inputs["edge_index"][1]).astype(np.int64)
    loop = np.arange(N, dtype=np.int64)
    src = np.concatenate([src, loop]); dst = np.concatenate([dst, loop])
    E = src.shape[0]

    nbins = NCORES * TPC
    deg = np.bincount(dst, minlength=NPAD).astype(np.int64)
    order = np.argsort(-deg, kind="stable")
    binof = np.empty(NPAD, dtype=np.int64)
    slotof = np.empty(NPAD, dtype=np.int64)
    pos = np.arange(NPAD)
    binof[order] = pos % nbins
    slotof[order] = pos // nbins
    new_id = binof * P + slotof

    ecore = binof[dst] // TPC         # owning core of each edge
    et = binof[dst] % TPC             # local tile
    er = new_id[src] // RSZ           # source range

    # per (core, tile, range) counts -> shared structure
    key = (ecore * TPC + et) * NRANGE + er
    cnt = np.bincount(key, minlength=NCORES * TPC * NRANGE).reshape(
        NCORES, TPC, NRANGE)
    nb = np.maximum(1, -(-cnt.max(axis=0) // P))   # [TPC, NRANGE] >=1
    nbs, chunks = make_structure([list(map(int, row)) for row in nb])
    gcol, chunk_info, JCOLS = col_layout(nbs, chunks)
    tcol, _ = tile_layout(nbs)

    # G-column order position for each (t, r): gcol[t][r][0]
    # slot assignment per core
    eorder = np.lexsort((er, et, ecore))
    src_s = src[eorder]; dst_s = dst[eorder]
    ecore_s = ecore[eorder]; et_s = et[eorder]; er_s = er[eorder]
    grpkey = (ecore_s * TPC + et_s) * NRANGE + er_s
    starts = np.zeros(NCORES * TPC * NRANGE + 1, dtype=np.int64)
    np.cumsum(np.bincount(grpkey, minlength=NCORES * TPC * NRANGE),
              out=starts[1:])
    eoff = np.arange(E) - starts[grpkey]

    # global G column (within core) and partition
    gbase = np.zeros((TPC, NRANGE), dtype=np.int64)
    for t in range(TPC):
        for r in range(NRANGE):
            gbase[t, r] = gcol[t][r][0]
    ecol = gbase[et_s, er_s] + eoff // P
    epart = eoff % P

    # tile-major column for dstloc
    tbase = np.array([s for (s, n) in tcol], dtype=np.int64)
    rcum = np.zeros((TPC, NRANGE), dtype=np.int64)
    for t in range(TPC):
        c = 0
        for r in range(NRANGE):
            rcum[t, r] = c; c += nbs[t][r]
    tcolidx = tbase[et_s] + rcum[et_s, er_s] + eoff // P

    idx_src = np.zeros((NCORES, P, JCOLS), dtype=np.int64)   # range-rel src row
    idx_dst = np.zeros((NCORES, P, JCOLS), dtype=np.int64)   # local tb row
    dstloc = np.full((NCORES, P, JCOLS), 200.0, dtype=np.float32)
    idx_src[ecore_s, epart, ecol] = new_id[src_s] % RSZ
    idx_dst[ecore_s, epart, ecol] = (et_s * P) + (new_id[dst_s] % P)
    dstloc[ecore_s, epart, tcolidx] = (new_id[dst_s] % P).astype(np.float32)

    # wrap indices for dma_gather: per (chunk, range-group) call, k=(col-local)*128+p
    # position [k%16, k//16] within the group's slice, replicated to 128 parts.
    def wrap(idx):
        out = np.zeros((NCORES, P, JCOLS * 8), dtype=np.int16)
        for (t0, t1), (cst, ncols, rspans) in zip(chunks, chunk_info):
            for (rs, re) in rspans:
                if re == rs:
                    continue
                blk = idx[:, :, rs:re]                       # [NC, P, nj]
                k = blk.transpose(0, 2, 1).reshape(NCORES, -1)  # k=(j,p)
                nk = k.shape[1]
                w = k.reshape(NCORES, nk // 16, 16).transpose(0, 2, 1)  # [NC,16,nk/16]
                wcols = np.tile(w, (1, 8, 1))                # replicate cores
                out[:, :, rs * 8:re * 8] = wcols
        return out

    idxw_src = wrap(idx_src)

    # dst-gather covers the whole chunk with one call (not per range):
    idxw_dst = np.zeros((NCORES, P, JCOLS * 8), dtype=np.int16)
    for (t0, t1), (cst, ncols, rspans) in zip(chunks, chunk_info):
        blk = idx_dst[:, :, cst:cst + ncols]
        k = blk.transpose(0, 2, 1).reshape(NCORES, -1)
        nk = k.shape[1]
        w = k.reshape(NCORES, nk // 16, 16).transpose(0, 2, 1)
        idxw_dst[:, :, cst * 8:(cst + ncols) * 8] = np.tile(w, (1, 8, 1))

    inv = np.empty(NPAD, dtype=np.int64)
    inv[new_id] = np.arange(NPAD)

    f32 = np.float32
    desT = np.ascontiguousarray(np.asarray(inputs["des"], f32).T)
    twT = np.ascontiguousarray(np.asarray(inputs["tweet"], f32).T)
    npT = np.ascontiguousarray(np.asarray(inputs["num_prop"], f32).T)
    cpT = np.ascontiguousarray(np.asarray(inputs["cat_prop"], f32).T)

    def slab(mT, c):
        cols = inv[c * NPC:(c + 1) * NPC]
        out = np.zeros((mT.shape[0], NPC), dtype=f32)
        real = cols < N
        out[:, real] = mT[:, cols[real]]
        return out.astype(bf)

    # ---- transforms ----
    a_s1 = np.asarray(inputs["gat1_asrc"], np.float64)  # [4, 32]
    a_d1 = np.asarray(inputs["gat1_adst"], np.float64)
    T1 = np.zeros((128, 128))
    for h in range(4):
        T1[h * 32:(h + 1) * 32, h * 32:(h + 1) * 32] = make_T(
            np.stack([a_s1[h], a_d1[h]]))
    T1i = np.linalg.inv(T1)
    a_s2 = np.asarray(inputs["gat2_asrc"], np.float64).reshape(128)
    a_d2 = np.asarray(inputs["gat2_adst"], np.float64).reshape(128)
    T2 = make_T(np.stack([a_s2, a_d2]))
    T2i = np.linalg.inv(T2)

    g1W = np.asarray(inputs["gat1_W"], np.float64)
    g2W = np.asarray(inputs["gat2_W"], np.float64)
    b1 = np.asarray(inputs["gat1_b"], np.float64)
    b2 = np.asarray(inputs["gat2_b"], np.float64)
    Wo1 = np.asarray(inputs["W_o1"], np.float64)
    bo1 = np.asarray(inputs["b_o1"], np.float64)

    gat1_Wp = g1W @ T1.T                       # table1 = T1 h1
    gat2_Wp = T1i.T @ g2W @ T2.T               # acts on x1' slab
    c2 = T2 @ (g2W.T @ b1)                     # bias inside table2
    W_o1p = T2i.T @ Wo1
    b_o1p = bo1 + Wo1.T @ b2

    b_enc = np.concatenate([np.asarray(inputs[k], f32) for k in
                            ("b_des", "b_tw", "b_np", "b_cp")]).reshape(128, 1)
    iota_bf = np.tile(np.arange(P, dtype=f32), (P, 1)).astype(bf)
    ident = np.eye(P, dtype=f32)

    in_maps = []
    for c in range(NCORES):
        m = {
            "desT": slab(desT, c), "tweetT": slab(twT, c),
            "numT": slab(npT, c), "catT": slab(cpT, c),
            "W_des": np.asarray(inputs["W_des"], f32).astype(bf),
            "W_tw": np.asarray(inputs["W_tw"], f32).astype(bf),
            "W_np": np.asarray(inputs["W_np"], f32).astype(bf),
            "W_cp": np.asarray(inputs["W_cp"], f32).astype(bf),
            "b_enc": b_enc,
            "W_in": np.asarray(inputs["W_in"], f32).astype(bf),
            "b_in": np.asarray(inputs["b_in"], f32).reshape(128, 1),
            "gat1_W": gat1_Wp.astype(f32).astype(bf),
            "gat2_W": gat2_Wp.astype(f32).astype(bf),
            "c2": c2.astype(f32).reshape(128, 1),
            "W_o1": W_o1p.astype(f32).astype(bf),
            "b_o1": b_o1p.astype(f32).reshape(128, 1),
            "W_o2": np.asarray(inputs["W_o2"], f32).astype(bf),
            "b_o2": np.asarray(inputs["b_o2"], f32).reshape(2, 1),
            "iota_bf": iota_bf, "ident_bf": ident.astype(bf),
            "idxw_src": idxw_src[c], "idxw_dst": idxw_dst[c],
            "dstloc": dstloc[c].astype(bf),
        }
        in_maps.append(m)
    struct = (nbs, chunks)
    return in_maps, {"new_id": new_id, "N": N}, struct


def assemble(results, meta):
    outT = np.concatenate([np.asarray(r["outT"]) for r in results], axis=1)
    return outT.T[meta["new_id"][:meta["N"]]].astype(np.float32)


_CACHE = {}

def _run(inputs, trace=False):
    in_maps, meta, struct = prep(inputs)
    if struct not in _CACHE:
        _CACHE[struct] = build(struct)
    from concourse.bass_utils import run_bass_kernel_spmd
    res = run_bass_kernel_spmd(_CACHE[struct], in_maps,
                               core_ids=list(range(NCORES)), trace=trace)
    return assemble(res.results, meta), res


def kernel(**inputs):
    out, _ = _run(inputs)
    return out
